# revision 1
# baseline (speedup 1.0000x reference)
"""Causal self-attention Trainium2 kernel (8 NeuronCores).

Sharding: tensor-parallel over heads x data-parallel over batch.
Core c handles batch b = c // 4 and head group g = c % 4 (4 heads of 16).
Each core computes q/k/v projections for its heads, causal attention, and a
partial output projection (its 256 columns of the 1024-wide contraction);
the host sums the 4 partials per batch.

Layout strategy (all transpose-free on device):
  - q,k are projected directly in transposed layout qkT[e, t] (e on
    partitions) so they feed the scores matmul as lhsT/rhs.
  - scores are computed transposed, sT[k_chunk=128, q_block=512], one
    matmul per (k_chunk, q_block) with K=hd=64.
  - softmax: no max-subtraction (scores ~ N(0,1), exp is safe in fp32);
    exp on ScalarE reading PSUM; causal mask added as -1e9 bias into PSUM
    for diagonal chunks; denominator comes free as an extra ones-column in
    the PV matmul's lhsT.
  - v is projected in natural layout v[t, hd] which is exactly the PV lhsT.
  - PV output yT[hd, q] is normalized via a K=1 broadcast matmul of the
    reciprocal row, then used directly as the proj lhsT.
All matmuls run as float32r (full PE rate at N>=256).
"""

import numpy as np

import concourse.bass as bass
from concourse import bacc
import concourse.mybir as mybir
import concourse.tile as tile
from concourse.bass_utils import run_bass_kernel_spmd

B, T, D, H = 2, 2048, 1024, 16
HD = D // H          # 64
HPC = 4              # heads per core
NCORES = 8
EQK = 2 * HPC * HD   # 512 rows of q+k per core
EV = HPC * HD        # 256 rows of v per core
TB = 512             # t/q block
NTB = T // TB        # 4
TC = 128             # t chunk
NTC = T // TC        # 16
DCH = D // 128       # 8 contraction chunks
F32 = mybir.dt.float32
F32R = mybir.dt.float32r

_cache = {}


def _ensure_ntff_hook():
    """The agent image's antenv lacks axon_hooks; fabricate it so
    run_bass_kernel_spmd(trace=True) can capture NTFF profiles."""
    import sys
    import types
    try:
        import antenv.axon_hooks  # noqa: F401
        return
    except ImportError:
        pass
    try:
        import antenv
        from trn_agent_boot.trn_boot import _ntff_profile_via_ctypes
        hook = {"h": _ntff_profile_via_ctypes("/opt/axon/libaxon_pjrt.so")}
        m = types.ModuleType("antenv.axon_hooks")
        m.get_axon_ntff_profile_hook = lambda: hook["h"]
        m.set_axon_ntff_profile_hook = lambda h: hook.update(h=h)
        sys.modules["antenv.axon_hooks"] = m
        antenv.axon_hooks = m
    except Exception:
        pass


def _build_nc():
    nc = bacc.Bacc("TRN2", target_bir_lowering=False, debug=False,
                  num_devices=NCORES)
    xT = nc.dram_tensor("xT", [D, T], F32R, kind="ExternalInput")
    wqk = nc.dram_tensor("wqk", [D, EQK], F32R, kind="ExternalInput")
    wv = nc.dram_tensor("wv", [D, EV], F32R, kind="ExternalInput")
    wp = nc.dram_tensor("wp", [EV, D], F32R, kind="ExternalInput")
    masks = nc.dram_tensor("masks", [4, 128, TB], F32, kind="ExternalInput")
    onesd = nc.dram_tensor("onesd", [128, HD], F32R, kind="ExternalInput")
    out = nc.dram_tensor("out", [T, D], F32, kind="ExternalOutput")

    with tile.TileContext(nc) as tc:
        with (
            nc.allow_low_precision(reason="fp32r matmul inputs; psum stays fp32"),
            tc.tile_pool(name="persist", bufs=1) as persist,
            tc.tile_pool(name="xin", bufs=2) as xin,
            tc.tile_pool(name="work", bufs=3) as work,
            tc.tile_pool(name="probsp", bufs=8) as probsp,
            tc.tile_pool(name="outp", bufs=3) as outp,
            tc.tile_pool(name="ps_big", bufs=4, space="PSUM") as ps_big,
            tc.tile_pool(name="ps_acc", bufs=4, space="PSUM") as ps_acc,
        ):
            # ---- persistent SBUF tensors ----
            wqk_sb = persist.tile([128, DCH, EQK], F32R)   # 16KB/part
            nc.sync.dma_start(wqk_sb[:], wqk.rearrange("(c p) e -> p c e", p=128))
            wv_sb = persist.tile([128, DCH, EV], F32R)     # 8KB/part
            nc.sync.dma_start(wv_sb[:], wv.rearrange("(c p) e -> p c e", p=128))
            wp_sb = persist.tile([128, 2, D], F32R)        # 8KB/part
            nc.sync.dma_start(wp_sb[:], wp.rearrange("(c p) e -> p c e", p=128))
            mask_sb = persist.tile([128, 4, TB], F32)     # 8KB/part
            nc.sync.dma_start(mask_sb[:], masks.rearrange("j p q -> p j q"))

            # qkT[e, t]: 4 chunks of 128 e-rows (q heads 01, q heads 23,
            # k heads 01, k heads 23), each [128, T]
            qkT = [persist.tile([128, T], F32R, tag=f"qkT{i}", name=f"qkT{i}")
                   for i in range(4)]
            # v_sb[t_chunk]: [128, h, 65]; col 64 of each head slot is 1.0
            v_sb = [persist.tile([128, HPC, HD + 1], F32R, tag=f"v{i}",
                                name=f"v{i}")
                    for i in range(NTC)]
            # yT: unnormalized-then-normalized attention output, [hd_all, t]
            yT = [persist.tile([128, T], F32R, tag=f"yT{i}", name=f"yT{i}")
                  for i in range(2)]

            def qT_ap(h):  # [64, T]
                return qkT[h // 2][64 * (h % 2):64 * (h % 2) + 64, :]

            def kT_ap(h):  # [64, T]
                return qkT[2 + h // 2][64 * (h % 2):64 * (h % 2) + 64, :]

            # ================= QKV projection =================
            for b in range(NTB):
                x_t = xin.tile([128, DCH, TB], F32R, tag="x")
                nc.sync.dma_start(
                    x_t[:], xT[:, b * TB:(b + 1) * TB]
                    .rearrange("(c p) t -> p c t", p=128))
                # q,k in transposed layout: psum[e_chunk 128, t 512]
                for ec in range(4):
                    ps = ps_big.tile([128, TB], F32, tag="mm", name="ps_qk")
                    for dc in range(DCH):
                        nc.tensor.matmul(
                            ps[:],
                            (wqk_sb[:, dc, 128 * ec:128 * (ec + 1)]),
                            (x_t[:, dc, :]),
                            start=(dc == 0), stop=(dc == DCH - 1))
                    nc.scalar.copy(qkT[ec][:, b * TB:(b + 1) * TB], ps[:])
                # v in natural layout: psum[t_chunk 128, hd 256]
                for t2 in range(4):
                    tc_i = 4 * b + t2
                    ps = ps_big.tile([128, TB], F32, tag="mm", name="ps_v")
                    for dc in range(DCH):
                        nc.tensor.matmul(
                            ps[:, 0:EV],
                            (x_t[:, dc, 128 * t2:128 * (t2 + 1)]),
                            (wv_sb[:, dc, :]),
                            start=(dc == 0), stop=(dc == DCH - 1))
                    nc.vector.tensor_copy(
                        v_sb[tc_i][:, :, 0:HD],
                        ps[:, 0:EV].rearrange("p (h f) -> p h f", h=HPC))
                    nc.sync.dma_start(v_sb[tc_i][:, :, HD], onesd[:, 0:HPC])

            # ================= attention =================
            # kc-outer / h-inner: PE sees 4 independent chains per round,
            # so each PV matmul's exp() has ~3 matmuls of latency cover.
            for b in range(NTB):
                nk = 4 * b + 4
                ps_pvs = [ps_acc.tile([HD + 1, TB], F32, tag="pv",
                                      name=f"pv_{b}_{h}") for h in range(HPC)]
                for kc in range(nk):
                    diag = kc >= 4 * b
                    # 4 sT matmuls back-to-back: head pairs live at base
                    # partitions 0/64 of their qkT chunk, so adjacent mms
                    # run concurrently in opposite PE row-groups.
                    ps_ss = []
                    for h in range(HPC):
                        ps_s = ps_big.tile([128, TB], F32, tag="mm",
                                           name=f"ps_s{h}")
                        nc.tensor.matmul(
                            ps_s[:],
                            (kT_ap(h)[:, 128 * kc:128 * (kc + 1)]),
                            (qT_ap(h)[:, b * TB:(b + 1) * TB]),
                            start=True, stop=True)
                        ps_ss.append(ps_s)
                    probss = []
                    for h in range(HPC):
                        probs = probsp.tile([128, TB], F32R, tag="probs",
                                            name=f"probs{h}")
                        nc.scalar.activation(
                            probs[:], ps_ss[h][:],
                            mybir.ActivationFunctionType.Exp,
                            scale=1.0 / np.sqrt(HD))
                        if diag:
                            nc.vector.tensor_mul(
                                probs[:], probs[:],
                                mask_sb[:, kc - 4 * b, :])
                        probss.append(probs)
                    for h in range(HPC):
                        nc.tensor.matmul(
                            ps_pvs[h][:],
                            (v_sb[kc][:, h, :]),
                            (probss[h][:]),
                            start=(kc == 0), stop=(kc == nk - 1))
                # tail: drain PSUM fast (frees pv slots), then normalize
                # yT in SBUF off the critical path.
                dens = []
                for h in range(HPC):
                    yslice = yT[h // 2][64 * (h % 2):64 * (h % 2) + 64,
                                        b * TB:(b + 1) * TB]
                    nc.vector.tensor_copy(yslice, ps_pvs[h][0:HD, :])
                    den = work.tile([1, TB], F32, tag="den", name=f"den{h}")
                    nc.scalar.copy(den[:], ps_pvs[h][HD:HD + 1, :])
                    dens.append(den)
                for h in range(HPC):
                    rec = work.tile([1, TB], F32, tag="rec", name=f"rec{h}")
                    nc.vector.reciprocal_approx_fast(rec[:], dens[h][:])
                    bc_sb = work.tile([128, TB], F32, tag="bc_sb")
                    nc.gpsimd.partition_broadcast(bc_sb[:], rec[:])
                    off = 64 * (h % 2)
                    yslice = yT[h // 2][off:off + 64,
                                        b * TB:(b + 1) * TB]
                    nc.vector.tensor_mul(yslice, yslice,
                                         bc_sb[off:off + 64, :])

            # ================= output projection =================
            for tc_i in range(NTC):
                for e in range(2):
                    ps = ps_big.tile([128, TB], F32, tag="mm", name="ps_proj")
                    for c in range(2):
                        nc.tensor.matmul(
                            ps[:],
                            (yT[c][:, 128 * tc_i:128 * (tc_i + 1)]),
                            (wp_sb[:, c, 512 * e:512 * (e + 1)]),
                            start=(c == 0), stop=(c == 1))
                    o_sb = outp.tile([128, TB], F32, tag="o")
                    nc.vector.tensor_copy(o_sb[:], ps[:])
                    nc.sync.dma_start(
                        out[128 * tc_i:128 * (tc_i + 1),
                            512 * e:512 * (e + 1)], o_sb[:])
    nc.compile()
    return nc


def _masks_np():
    m = np.zeros((4, 128, TB), dtype=np.float32)
    kr = np.arange(128)[:, None]
    qc = np.arange(TB)[None, :]
    for j in range(4):
        m[j] = np.where(kr <= qc - 128 * j, 1.0, 0.0).astype(np.float32)
    return m


def _prep_in_maps(x, w_qkv, w_proj):
    masks = _masks_np()
    in_maps = []
    for c in range(NCORES):
        b, g = c // 4, c % 4
        heads = slice(g * HPC * HD, (g + 1) * HPC * HD)      # 256 rows
        wq = w_qkv[0 * D:1 * D][heads]                        # [256, 1024]
        wk = w_qkv[1 * D:2 * D][heads]
        wv = w_qkv[2 * D:3 * D][heads]
        in_maps.append({
            "xT": np.ascontiguousarray(x[b].T),               # [1024, 2048]
            "wqk": np.ascontiguousarray(
                np.concatenate([wq, wk], axis=0).T),          # [1024, 512]
            "wv": np.ascontiguousarray(wv.T),                 # [1024, 256]
            "wp": np.ascontiguousarray(w_proj[:, heads].T),   # [256, 1024]
            "masks": masks,
            "onesd": np.ones((128, HD), dtype=np.float32),
        })
    return in_maps


def kernel(x, w_qkv, w_proj, _trace=False):
    x = np.asarray(x, dtype=np.float32)
    w_qkv = np.asarray(w_qkv, dtype=np.float32)
    w_proj = np.asarray(w_proj, dtype=np.float32)
    if _trace:
        _ensure_ntff_hook()
    if "nc" not in _cache:
        _cache["nc"] = _build_nc()
    nc = _cache["nc"]
    in_maps = _prep_in_maps(x, w_qkv, w_proj)
    res = run_bass_kernel_spmd(nc, in_maps, list(range(NCORES)),
                               trace=_trace)
    out = np.zeros((B, T, D), dtype=np.float32)
    for c in range(NCORES):
        out[c // 4] += res.results[c]["out"]
    if _trace:
        _cache["last_result"] = res
    return out



# revision 3
# speedup vs baseline: 1.6508x; 1.6508x over previous
"""Causal self-attention Trainium2 kernel (8 NeuronCores).

Sharding: tensor-parallel over heads x data-parallel over batch.
Core c handles batch b = c // 4 and head group g = c % 4 (4 heads of 16).
Each core computes q/k/v projections for its heads, causal attention, and a
partial output projection (its 256 columns of the 1024-wide contraction);
the host sums the 4 partials per batch.

v2 design (bf16 matmul operands, ACT-bound attention):
  - All matmul inputs are bf16 (PSUM accumulation stays fp32): 2x moving
    rate + FWL weight loads vs the fp32r baseline.
  - q,k projected directly transposed qkT[e, t]; v in natural v[t, hd]
    with a ones column for the softmax denominator (PV lhsT trick).
  - Attention runs per head-PAIR so each exp() ACTIVATE covers 2 heads'
    score banks in one instruction ((N+352)-cycle cost amortized) and only
    2 PV accumulator banks are live at a time (PSUM: 2x2-bank score slots
    + 4 PV banks = 8).
  - Causal trim: diagonal chunk j only computes q-columns >= 128*j; the
    remaining partial triangle is masked with one [128,128] bf16 multiply
    on DVE.
  - Scores per pair run concurrently in opposite PE row groups (heads at
    base partitions 0/64, K=64).
  - Emission is software-pipelined: scores(r+1) is emitted before PV(r) so
    the PE never FIFO-stalls behind an exp.
  - Normalization is fused into the PV drain: yT = psum * bcast(1/den).
"""

import numpy as np
import ml_dtypes

import concourse.bass as bass
from concourse import bacc
import concourse.mybir as mybir
import concourse.tile as tile
from concourse.bass_utils import run_bass_kernel_spmd

B, T, D, H = 2, 2048, 1024, 16
HD = D // H          # 64
HPC = 4              # heads per core
NCORES = 8
EQK = 2 * HPC * HD   # 512 rows of q+k per core
EV = HPC * HD        # 256 rows of v per core
TB = 512             # t/q block
NTB = T // TB        # 4
TC = 128             # t chunk
NTC = T // TC        # 16
DCH = D // 128       # 8 contraction chunks
F32 = mybir.dt.float32
BF16 = mybir.dt.bfloat16

_cache = {}


def _ensure_ntff_hook():
    """The agent image's antenv lacks axon_hooks; fabricate it so
    run_bass_kernel_spmd(trace=True) can capture NTFF profiles."""
    import sys
    import types
    try:
        import antenv.axon_hooks  # noqa: F401
        return
    except ImportError:
        pass
    try:
        import antenv
        from trn_agent_boot.trn_boot import _ntff_profile_via_ctypes
        hook = {"h": _ntff_profile_via_ctypes("/opt/axon/libaxon_pjrt.so")}
        m = types.ModuleType("antenv.axon_hooks")
        m.get_axon_ntff_profile_hook = lambda: hook["h"]
        m.set_axon_ntff_profile_hook = lambda h: hook.update(h=h)
        sys.modules["antenv.axon_hooks"] = m
        antenv.axon_hooks = m
    except Exception:
        pass


def _build_nc():
    nc = bacc.Bacc("TRN2", target_bir_lowering=False, debug=False,
                  num_devices=NCORES)
    xT = nc.dram_tensor("xT", [D, T], BF16, kind="ExternalInput")
    wqk = nc.dram_tensor("wqk", [D, EQK], BF16, kind="ExternalInput")
    wv = nc.dram_tensor("wv", [D, EV], BF16, kind="ExternalInput")
    wp = nc.dram_tensor("wp", [EV, D], BF16, kind="ExternalInput")
    tri = nc.dram_tensor("tri", [128, 128], BF16, kind="ExternalInput")
    out = nc.dram_tensor("out", [T, D], F32, kind="ExternalOutput")

    with tile.TileContext(nc) as tc:
        with (
            nc.allow_low_precision(reason="bf16 matmul inputs; psum stays fp32"),
            tc.tile_pool(name="persist", bufs=1) as persist,
            tc.tile_pool(name="xin", bufs=2) as xin,
            tc.tile_pool(name="work", bufs=4) as work,
            tc.tile_pool(name="probsp", bufs=4) as probsp,
            tc.tile_pool(name="outp", bufs=3) as outp,
            tc.tile_pool(name="ps_mm", bufs=2, space="PSUM") as ps_mm,
            tc.tile_pool(name="ps_pv", bufs=4, space="PSUM") as ps_pv,
        ):
            # ---- persistent SBUF tensors ----
            wqk_sb = persist.tile([128, DCH, EQK], BF16)   # 8KB/part
            nc.sync.dma_start(wqk_sb[:], wqk.rearrange("(c p) e -> p c e", p=128))
            wv_sb = persist.tile([128, DCH, EV], BF16)     # 4KB/part
            nc.sync.dma_start(wv_sb[:], wv.rearrange("(c p) e -> p c e", p=128))
            wp_sb = persist.tile([128, 2, D], BF16)        # 4KB/part
            nc.sync.dma_start(wp_sb[:], wp.rearrange("(c p) e -> p c e", p=128))
            tri_sb = persist.tile([128, 128], BF16)
            nc.sync.dma_start(tri_sb[:], tri[:, :])

            # qkT[p, c, t]: c in {0: q h01, 1: q h23, 2: k h01, 3: k h23};
            # head pair member at base partition 0/64.
            qkT = persist.tile([128, 4, T], BF16, name="qkT")    # 16KB/part
            # v_sb[t_chunk]: [128, h, 65]; col 64 of each head slot is 1.0
            v_sb = [persist.tile([128, HPC, HD + 1], BF16, tag=f"v{i}",
                                name=f"v{i}")
                    for i in range(NTC)]
            # yT: normalized attention output, [p, c, t]; c=0 heads01, 1 h23
            yT = persist.tile([128, 2, T], BF16, name="yT")      # 8KB/part

            # ones columns for the denominator trick
            for i in range(NTC):
                nc.gpsimd.memset(v_sb[i][:, :, HD], 1.0)

            def emit_qkv(b):
                x_t = xin.tile([128, DCH, TB], BF16, tag="x")
                nc.sync.dma_start(
                    x_t[:], xT[:, b * TB:(b + 1) * TB]
                    .rearrange("(c p) t -> p c t", p=128))
                # q,k transposed: psum[e_chunk 128, t 512], ec pairs share a slot
                for pr in range(2):
                    ps = ps_mm.tile([128, 2, TB], F32, tag="mm", name="ps_qk")
                    for sub in range(2):
                        ec = 2 * pr + sub
                        for dc in range(DCH):
                            nc.tensor.matmul(
                                ps[:, sub, :],
                                wqk_sb[:, dc, 128 * ec:128 * (ec + 1)],
                                x_t[:, dc, :],
                                start=(dc == 0), stop=(dc == DCH - 1))
                    nc.vector.tensor_copy(
                        qkT[:, 2 * pr:2 * pr + 2, b * TB:(b + 1) * TB], ps[:])
                # v natural: psum[t_chunk 128, hd 256], two t2 per slot
                for pr in range(2):
                    ps = ps_mm.tile([128, 2, TB], F32, tag="mm", name="ps_v")
                    for sub in range(2):
                        t2 = 2 * pr + sub
                        for dc in range(DCH):
                            nc.tensor.matmul(
                                ps[:, sub, 0:EV],
                                x_t[:, dc, 128 * t2:128 * (t2 + 1)],
                                wv_sb[:, dc, :],
                                start=(dc == 0), stop=(dc == DCH - 1))
                    for sub in range(2):
                        tc_i = 4 * b + 2 * pr + sub
                        nc.vector.tensor_copy(
                            v_sb[tc_i][:, :, 0:HD],
                            ps[:, sub, 0:EV].rearrange("p (h f) -> p h f",
                                                       h=HPC))

            def emit_scores(b, pr, kc):
                """Scores for head pair pr at (q block b, k chunk kc).
                Returns (ps, q0)."""
                j = kc - 4 * b
                q0 = 128 * j if j >= 0 else 0
                ps = ps_mm.tile([128, 2, TB], F32, tag="mm", name="ps_s")
                for sub in range(2):
                    nc.tensor.matmul(
                        ps[:, sub, q0:],
                        qkT[64 * sub:64 * sub + 64, 2 + pr,
                            128 * kc:128 * (kc + 1)],
                        qkT[64 * sub:64 * sub + 64, pr,
                            b * TB + q0:(b + 1) * TB],
                        start=True, stop=True)
                return ps, q0

            def emit_exp(b, pr, kc, ps, q0):
                """exp over both heads' banks; triangle mask on diagonal."""
                diag = kc >= 4 * b
                probs = probsp.tile([128, 2, TB], BF16, tag="probs")
                nc.scalar.activation(
                    probs[:, :, q0:], ps[:, :, q0:],
                    mybir.ActivationFunctionType.Exp,
                    scale=1.0 / np.sqrt(HD))
                if diag:
                    for sub in range(2):
                        nc.vector.tensor_mul(
                            probs[:, sub, q0:q0 + 128],
                            probs[:, sub, q0:q0 + 128],
                            tri_sb[:])
                return probs

            def emit_pv(b, pr, kc, probs, q0, pvs):
                nk = 4 * b + 4
                for sub in range(2):
                    h = 2 * pr + sub
                    nc.tensor.matmul(
                        pvs[sub][:, q0:],
                        v_sb[kc][:, h, :],
                        probs[:, sub, q0:],
                        start=(kc == 0), stop=(kc == nk - 1))

            def emit_norm(b, pr, pvs):
                for sub in range(2):
                    h = 2 * pr + sub
                    den = work.tile([1, TB], F32, tag="den")
                    nc.vector.tensor_copy(den[:], pvs[sub][HD:HD + 1, :])
                    rec = work.tile([1, TB], F32, tag="rec")
                    nc.vector.reciprocal_approx_fast(rec[:], den[:])
                    bc = work.tile([64, TB], F32, tag="bc")
                    nc.gpsimd.partition_broadcast(bc[:], rec[:])
                    nc.vector.tensor_mul(
                        yT[64 * sub:64 * sub + 64, pr,
                           b * TB:(b + 1) * TB],
                        pvs[sub][0:HD, :], bc[:])

            def emit_attn_pair(b, pr):
                """Full softmax-attention pass for head pair pr of block b,
                software-pipelined: scores(r+1) emitted before PV(r)."""
                nk = 4 * b + 4
                pvs = [ps_pv.tile([HD + 1, TB], F32, tag="pv",
                                  name=f"pv{b}_{pr}_{s}") for s in range(2)]
                ps, q0 = emit_scores(b, pr, 0)
                pend = (0, ps, q0)
                for kc in range(1, nk):
                    pkc, pps, pq0 = pend
                    probs = emit_exp(b, pr, pkc, pps, pq0)
                    ps, q0 = emit_scores(b, pr, kc)
                    emit_pv(b, pr, pkc, probs, pq0, pvs)
                    pend = (kc, ps, q0)
                pkc, pps, pq0 = pend
                probs = emit_exp(b, pr, pkc, pps, pq0)
                emit_pv(b, pr, pkc, probs, pq0, pvs)
                emit_norm(b, pr, pvs)

            def emit_proj(b):
                for tq in range(4):
                    tc_i = 4 * b + tq
                    ps = ps_mm.tile([128, 2, TB], F32, tag="mm", name="ps_pj")
                    for e in range(2):
                        for c in range(2):
                            nc.tensor.matmul(
                                ps[:, e, :],
                                yT[:, c, 128 * tc_i:128 * (tc_i + 1)],
                                wp_sb[:, c, 512 * e:512 * (e + 1)],
                                start=(c == 0), stop=(c == 1))
                    o_sb = outp.tile([128, 2, TB], F32, tag="o")
                    nc.vector.tensor_copy(o_sb[:], ps[:])
                    nc.sync.dma_start(
                        out[128 * tc_i:128 * (tc_i + 1), :],
                        o_sb.rearrange("p a b -> p (a b)"))

            for b in range(NTB):
                emit_qkv(b)
                for pr in range(2):
                    emit_attn_pair(b, pr)
                emit_proj(b)
    nc.compile()
    return nc


def _tri_np():
    # tri[k, q] = 1 where k <= q (block-local causal keep mask)
    kr = np.arange(128)[:, None]
    qc = np.arange(128)[None, :]
    return (kr <= qc).astype(ml_dtypes.bfloat16)


def _prep_in_maps(x, w_qkv, w_proj):
    bf = ml_dtypes.bfloat16
    tri = _tri_np()
    in_maps = []
    for c in range(NCORES):
        b, g = c // 4, c % 4
        heads = slice(g * HPC * HD, (g + 1) * HPC * HD)      # 256 rows
        wq = w_qkv[0 * D:1 * D][heads]                        # [256, 1024]
        wk = w_qkv[1 * D:2 * D][heads]
        wvm = w_qkv[2 * D:3 * D][heads]
        in_maps.append({
            "xT": np.ascontiguousarray(x[b].T).astype(bf),    # [1024, 2048]
            "wqk": np.ascontiguousarray(
                np.concatenate([wq, wk], axis=0).T).astype(bf),  # [1024, 512]
            "wv": np.ascontiguousarray(wvm.T).astype(bf),     # [1024, 256]
            "wp": np.ascontiguousarray(w_proj[:, heads].T).astype(bf),
            "tri": tri,
        })
    return in_maps


def kernel(x, w_qkv, w_proj, _trace=False):
    x = np.asarray(x, dtype=np.float32)
    w_qkv = np.asarray(w_qkv, dtype=np.float32)
    w_proj = np.asarray(w_proj, dtype=np.float32)
    if _trace:
        _ensure_ntff_hook()
    if "nc" not in _cache:
        _cache["nc"] = _build_nc()
    nc = _cache["nc"]
    in_maps = _prep_in_maps(x, w_qkv, w_proj)
    res = run_bass_kernel_spmd(nc, in_maps, list(range(NCORES)),
                               trace=_trace)
    out = np.zeros((B, T, D), dtype=np.float32)
    for c in range(NCORES):
        out[c // 4] += res.results[c]["out"]
    if _trace:
        _cache["last_result"] = res
    return out
